# revision 1
# baseline (speedup 1.0000x reference)
"""Trainium2 Bass kernel for nn_EquivariantLayer (GNN message passing).

Computation:
    lw  = weights[:, ROT] reshaped            (1280, 512), ROT closed-form
    msg = conn_vals[:, None] * x[conn_cols]   (NNZ, 16)
    agg = segment_sum(msg, conn_rows)         (N*80, 16)
    out = agg.reshape(N, 1280) @ lw           (N, 512)

Distribution: edges sharded by destination vertex across 8 NeuronCores
(2500 vertices / 200k destination rows each); x replicated.

Device pipeline (per core):
  * Edges are split into 4 sublists by bin%4 and row-sorted; chunks of 128
    edges aggregate straight into h^T-layout PSUM windows
    [partition = 32*(b%4)+c, free = (v_local, b//4)] via small matmuls
    (lhsT = gathered-and-scaled messages [128 slots x 16 feat],
    rhs = 0/1 destination-cell mask [128 x 48]) — the four sublists write
    disjoint 32-partition strips, giving PE column-group concurrency and
    eliminating any h staging traffic.
  * The per-edge gather x[cols] runs on GPSIMD (ap_gather, fp32, 8 Q7
    cores); ACT squeezes fp32->fp16; the DMA xbar transposes
    [feat x slots] -> [slots x feat] in bulk (dma_start_transpose, 3D out).
  * Masks are built at DVE 2x rate via a static jj-table is_equal against
    per-chunk cell offsets (g-innermost layout keeps all operands packed);
    conn_vals are folded into the messages with one broadcast multiply.
  * PSUM windows accumulate over ~64 chunks (has_written handles partial
    coverage; untouched strips stay zero from a one-time buffer memset);
    ACT copies each finished window into h^T fp16, and a 20-block fp16
    dense matmul (h^T stationary, lw moving) produces y per 125 vertices.
"""

import numpy as np

# ---------------------------------------------------------------- constants
P_, T_ = 5, 16
B = 80
C_IN = 16
N_VERTS = 20000
K = 1280
N_CORES = 8
NV = N_VERTS // N_CORES   # 2500 vertices per core
WV = 25                   # vertices per psum window
NWIN = NV // WV           # 100
NBH = 20                  # bin // 4
NCELL = WV * NBH          # 500 psum columns per window
MW = 48                   # mask width (cells per chunk)
CH = 128                  # slots per chunk
VB = 125                  # vertices per dense block (5 windows)
NVB = NV // VB            # 20
O = 512
GRP_CH = 64               # chunks per group (8 transpose tiles)


# ------------------------------------------------------------- host prep
def _build_template(rows):
    """Shared chunk template across the 8 cores.

    Returns (chunk_meta [(w, bl2, j0)], win_ranges [(c0, c1)] per window,
    core_slots [per core: list of (p0, p1) slices into `order` or None],
    order [global edge sort permutation]).
    """
    v_all = rows // B
    core = v_all // NV
    b_all = rows % B
    vloc = v_all % NV
    w_all = vloc // WV
    bl2_all = b_all % 4
    j_all = (vloc % WV) * NBH + b_all // 4

    order = np.lexsort((j_all, bl2_all, w_all, core))
    so_j = j_all[order]
    keys = ((core[order].astype(np.int64) * NWIN) + w_all[order]) * 4 + bl2_all[order]
    starts = np.searchsorted(keys, np.arange(N_CORES * NWIN * 4 + 1))

    chunk_meta = []
    win_ranges = []
    core_slots = [[] for _ in range(N_CORES)]

    for w in range(NWIN):
        wstart = len(chunk_meta)
        ptr = np.zeros((N_CORES, 4), np.int64)
        lim = np.zeros((N_CORES, 4), np.int64)
        for d in range(N_CORES):
            for bl in range(4):
                kidx = (d * NWIN + w) * 4 + bl
                ptr[d, bl] = starts[kidx]
                lim[d, bl] = starts[kidx + 1]
        sub_chunks = [[] for _ in range(4)]
        for bl in range(4):
            prev_j0 = None
            while True:
                active = [d for d in range(N_CORES) if ptr[d, bl] < lim[d, bl]]
                if not active:
                    if prev_j0 is None:
                        prev_j0 = 0
                        sub_chunks[bl].append((0, None))
                    while prev_j0 < NCELL - MW:
                        prev_j0 = min(prev_j0 + MW, NCELL - MW)
                        sub_chunks[bl].append((prev_j0, None))
                    break
                j0 = int(min(so_j[ptr[d, bl]] for d in active))
                if prev_j0 is not None:
                    j0 = min(max(j0, prev_j0), prev_j0 + MW)
                else:
                    j0 = 0
                j0 = min(j0, NCELL - MW)
                takes = []
                for d in range(N_CORES):
                    p0 = ptr[d, bl]
                    p1 = min(lim[d, bl],
                             p0 + np.searchsorted(so_j[p0:lim[d, bl]], j0 + MW),
                             p0 + CH)
                    takes.append((int(p0), int(p1)))
                    ptr[d, bl] = p1
                sub_chunks[bl].append((j0, takes))
                prev_j0 = j0
        maxc = max(len(sub_chunks[bl]) for bl in range(4))
        for i in range(maxc):
            for bl in range(4):
                if i < len(sub_chunks[bl]):
                    j0, takes = sub_chunks[bl][i]
                    chunk_meta.append((w, bl, j0))
                    for d in range(N_CORES):
                        core_slots[d].append(None if takes is None else takes[d])
        win_ranges.append((wstart, len(chunk_meta)))

    while len(chunk_meta) % GRP_CH != 0:
        chunk_meta.append((NWIN - 1, 0, NCELL - MW))
        for d in range(N_CORES):
            core_slots[d].append(None)
    s, _ = win_ranges[-1]
    win_ranges[-1] = (s, len(chunk_meta))
    return chunk_meta, win_ranges, core_slots, order


def _build_core_arrays(chunk_meta, core_slots_d, order, rows, cols, vals):
    """idx16 [128, NC] int16, valsb [128, NC] fp16, rl [128, NC] fp16."""
    v_all = rows // B
    b_all = rows % B
    j_full = ((v_all % NV) % WV) * NBH + b_all // 4

    NC = len(chunk_meta)
    idx16 = np.zeros((128, NC), np.int16)
    valsb = np.zeros((128, NC), np.float16)
    rl = np.full((128, NC), -1.0, np.float16)

    for c, take in enumerate(core_slots_d):
        if take is None:
            continue
        p0, p1 = take
        n = p1 - p0
        if n == 0:
            continue
        w, bl, j0 = chunk_meta[c]
        g = c % 8
        t = c // 8
        eids = order[p0:p1]
        ss = np.arange(n)
        idx16[16 * g + ss % 16, t * 8 + ss // 16] = cols[eids].astype(np.int16)
        valsb[ss, c] = vals[eids].astype(np.float16)
        rl[ss, c] = (j_full[eids] - j0).astype(np.float16)
    return idx16, valsb, rl


def _lw_from_weights(weights):
    k = np.arange(K)
    ci, u = k // B, k % B
    t, p = u // P_, u % P_
    j = np.arange(16)
    rot = ci[:, None] * B + ((t[:, None] + j) % T_) * P_ + p[:, None]
    lw = weights[:, rot]
    return np.transpose(lw, (1, 0, 2)).reshape(K, 32 * 16)


def _lw_device_layout(lw):
    """(1280, 512) -> [128, NBH*512]; partition 32*bl2+c, block bh (k=b*16+c)."""
    out = np.zeros((128, NBH * O), np.float32)
    for bl2 in range(4):
        for c in range(C_IN):
            p = 32 * bl2 + c
            for bh in range(NBH):
                k = (bh * 4 + bl2) * C_IN + c
                out[p, bh * O:(bh + 1) * O] = lw[k]
    return out


# ------------------------------------------------------------ bass program
def _build_program(chunk_meta, win_ranges):
    from concourse import bacc, mybir, tile
    import concourse.bass as bass

    NC = len(chunk_meta)
    NGRP = NC // GRP_CH

    nc = bacc.Bacc("TRN2", target_bir_lowering=False, debug=False,
                   num_devices=N_CORES)
    f32 = mybir.dt.float32
    f16 = mybir.dt.float16
    i16 = mybir.dt.int16

    xT_in = nc.dram_tensor("xT", [128, N_VERTS], f32, kind="ExternalInput")
    lw_in = nc.dram_tensor("lw", [128, NBH * O], f16, kind="ExternalInput")
    idx_in = nc.dram_tensor("idx16", [128, NC], i16, kind="ExternalInput")
    vals_in = nc.dram_tensor("vals", [128, NC], f16, kind="ExternalInput")
    rl_in = nc.dram_tensor("rl", [128, NC], f16, kind="ExternalInput")
    t48_in = nc.dram_tensor("t48", [128, MW * 8], f16, kind="ExternalInput")
    y_out = nc.dram_tensor("y", [NV, O], f32, kind="ExternalOutput")

    win_first = {w: r[0] for w, r in enumerate(win_ranges)}
    win_last = {w: r[1] - 1 for w, r in enumerate(win_ranges)}

    with tile.TileContext(nc) as tc:
        with (
            tc.tile_pool(name="persist", bufs=1) as pp,
            tc.tile_pool(name="raw", bufs=2) as rawp,
            tc.tile_pool(name="rawb", bufs=3) as rawbp,
            tc.tile_pool(name="msgt", bufs=3) as msgtp,
            tc.tile_pool(name="msgtv", bufs=3) as msgtvp,
            tc.tile_pool(name="mask", bufs=2) as maskp,
            tc.tile_pool(name="ht", bufs=2) as htp,
            tc.tile_pool(name="outp", bufs=2) as outp,
            tc.tile_pool(name="wps", bufs=3, space="PSUM") as wps,
            tc.tile_pool(name="ops", bufs=2, space="PSUM") as ops,
        ):
            xT = pp.tile([128, N_VERTS], f32)
            nc.sync.dma_start(xT[:], xT_in[:])
            lw_sb = pp.tile([128, NBH * O], f16)
            nc.sync.dma_start(lw_sb[:], lw_in[:])
            idx_sb = pp.tile([128, NC], i16)
            nc.sync.dma_start(idx_sb[:], idx_in[:])
            vals_sb = pp.tile([128, NC], f16)
            nc.sync.dma_start(vals_sb[:], vals_in[:])
            rl_sb = pp.tile([128, NC], f16)
            nc.sync.dma_start(rl_sb[:], rl_in[:])
            t48_sb = pp.tile([128, MW * 8], f16)
            nc.sync.dma_start(t48_sb[:], t48_in[:])

            wtiles = {}
            httiles = {}
            for gi in range(NGRP):
                c0 = gi * GRP_CH
                raw_t = rawp.tile([128, 1024], f32, tag="raw")
                nc.gpsimd.ap_gather(
                    raw_t[:], xT[:], idx_sb[:, c0:c0 + GRP_CH],
                    channels=128, num_elems=N_VERTS, d=1, num_idxs=1024)
                rawb_t = rawbp.tile([128, 1024], f16, tag="rawb")
                nc.scalar.copy(rawb_t[:], raw_t[:])
                msgt_t = msgtp.tile([128, 1024], f16, tag="msgt")
                nc.sync.dma_start_transpose(
                    msgt_t[:].rearrange("p (t f) -> p t f", t=8), rawb_t[:])
                msgtv_t = msgtvp.tile([128, 1024], f16, tag="msgtv")
                nc.vector.tensor_tensor(
                    out=msgtv_t[:].rearrange("p (t g c) -> p t g c", t=8, g=8),
                    in0=msgt_t[:].rearrange("p (t g c) -> p t g c", t=8, g=8),
                    in1=vals_sb[:, c0:c0 + GRP_CH]
                        .rearrange("p (t g) -> p t g", t=8)
                        .unsqueeze(3).broadcast_to([128, 8, 8, C_IN]),
                    op=mybir.AluOpType.mult)
                mask_t = maskp.tile([128, 8 * MW * 8], f16, tag="mask")
                nc.vector.tensor_tensor(
                    out=mask_t[:].rearrange("p (t j g) -> p t j g", t=8, j=MW),
                    in0=t48_sb[:].rearrange("p (j g) -> p j g", j=MW)
                        .unsqueeze(1).broadcast_to([128, 8, MW, 8]),
                    in1=rl_sb[:, c0:c0 + GRP_CH]
                        .rearrange("p (t g) -> p t g", t=8)
                        .unsqueeze(2).broadcast_to([128, 8, MW, 8]),
                    op=mybir.AluOpType.is_equal)
                mask_r = mask_t[:].rearrange("p (t j g) -> p t j g", t=8, j=MW)

                for cl in range(GRP_CH):
                    c = c0 + cl
                    w, bl2, j0 = chunk_meta[c]
                    t, g = cl // 8, cl % 8
                    if c == win_first[w]:
                        wtiles[w] = wps.tile([128, NCELL], f32, tag="wps",
                                             name=f"w{w}")
                        # psum banks are recycled with stale has_written bits;
                        # accumulate-onto-zero is correct in either bit state.
                        nc.vector.memset(wtiles[w][:], 0.0)
                    nc.tensor.matmul(
                        wtiles[w][32 * bl2:32 * bl2 + 16, j0:j0 + MW],
                        lhsT=msgtv_t[:, cl * 16:(cl + 1) * 16],
                        rhs=mask_r[:, t, :, g],
                        start=False, stop=(c == win_last[w]),
                        tile_position=(0, 32 * bl2))
                    if c == win_last[w]:
                        vb, wi = w // 5, w % 5
                        if wi == 0:
                            httiles[vb] = htp.tile([128, NBH * VB], f16,
                                                   tag="ht", name=f"ht{vb}")
                        nc.scalar.copy(
                            httiles[vb][:]
                            .rearrange("p (bh v) -> p bh v", bh=NBH)
                            [:, :, wi * WV:(wi + 1) * WV],
                            wtiles[w][:].rearrange("p (v bh) -> p bh v", v=WV))
                        del wtiles[w]
                        if wi == 4:
                            ht = httiles.pop(vb)
                            opsum = ops.tile([128, O], f32, tag="op",
                                             name=f"op{vb}")
                            for bh in range(NBH):
                                nc.tensor.matmul(
                                    opsum[0:VB, :],
                                    lhsT=ht[:, bh * VB:(bh + 1) * VB],
                                    rhs=lw_sb[:, bh * O:(bh + 1) * O],
                                    start=(bh == 0), stop=(bh == NBH - 1))
                            out_sb = outp.tile([128, O], f32, tag="osb",
                                               name=f"osb{vb}")
                            nc.scalar.copy(out_sb[0:VB, :], opsum[0:VB, :])
                            nc.sync.dma_start(y_out[vb * VB:(vb + 1) * VB, :],
                                              out_sb[0:VB, :])
    nc.compile()
    return nc


# ---------------------------------------------------------------- kernel
def kernel(x, weights, conn_vals, conn_rows, conn_cols):
    import sys
    for p in ("/opt/trn_rl_repo",):
        if p not in sys.path:
            sys.path.append(p)
    from concourse.bass_utils import run_bass_kernel_spmd

    x = np.asarray(x, dtype=np.float32)
    weights = np.asarray(weights, dtype=np.float32)
    conn_vals = np.asarray(conn_vals, dtype=np.float32)
    rows = np.asarray(conn_rows).astype(np.int64)
    cols = np.asarray(conn_cols).astype(np.int64)

    chunk_meta, win_ranges, core_slots, order = _build_template(rows)

    nc = _build_program(chunk_meta, win_ranges)

    xT = np.ascontiguousarray(np.tile(x.T, (8, 1)))              # [128, 20000]
    lw_arr = np.ascontiguousarray(
        _lw_device_layout(_lw_from_weights(weights))).astype(np.float16)
    t48 = np.ascontiguousarray(
        np.broadcast_to(
            np.repeat(np.arange(MW, dtype=np.float16), 8)[None, :],
            (128, MW * 8)))

    in_maps = []
    for d in range(N_CORES):
        idx16, valsb, rl = _build_core_arrays(
            chunk_meta, core_slots[d], order, rows, cols, conn_vals)
        in_maps.append({
            "xT": xT, "lw": lw_arr, "idx16": idx16,
            "vals": valsb, "rl": rl, "t48": t48,
        })

    res = run_bass_kernel_spmd(nc, in_maps, core_ids=list(range(N_CORES)),
                               trace=bool(globals().get("TRACE", False)))
    global LAST_EXEC_NS, LAST_RESULTS
    LAST_EXEC_NS = res.exec_time_ns
    LAST_RESULTS = res
    out = np.concatenate([res.results[d]["y"] for d in range(N_CORES)], axis=0)
    return out.astype(np.float32)


if __name__ == "__main__":
    pass

